# revision 17
# baseline (speedup 1.0000x reference)
"""Llama-style GQA attention (S=4096, H=2048, 16 q heads / 4 kv heads, d=128, fp32)
on 8 Trainium2 NeuronCores.

Sharding: 4 head-groups x 2 sequence-halves. Core c = 2*g + sh owns q heads
[4g, 4g+4) (one kv head g) and query rows [2048*sh, 2048*(sh+1)). Each core
computes its partial o_proj output transposed ([out_feat, seq], bf16); the host
sums the 4 head-group partials per sequence half and concatenates.

v2 schedule (single pass over hs, ACT-paced attention inner loop):
  hs chunk order per core: OTHER seq-half first (kr/vsb cols 0..2048), own
  half last (cols 2048..4096) -> rope tables for q are cos_k[:, 2048+q].
  proj: stream hs in 512-col chunks (pairs share [128,1024] PSUM tiles);
        K/V projected for all 8 chunks, Q only for own chunks 0,1 (block b0);
        Q for chunks 2,3 (block b1) is deferred and drip-fed into attention
        block b0 through the psX PSUM ring.
  attn: per (block, head): 32 key-tiles; each jt issues
        scoresT MM (2x512) -> ACT exp [128,1024] -> PV MM (2x512, PSUM accum)
        -> DVE 4x-mode prob accumulate, plus one deferred PE item (Q proj MM
        for b0, or a 4-MM o_proj group every 4th jt for b1). Denominator =
        GpSimd partition_all_reduce; 1/x via DVE reciprocal_approx_fast;
        PSUM attn accumulator freed early via GpSimd bf16 copy; o_proj
        outputs copied bf16 on GpSimd and DMA'd out per 512-col group.
"""

import math

import numpy as np
import ml_dtypes

_S, _H, _HD = 4096, 2048, 128
_NCORES = 8
_SQ = _S // 2          # per-core query rows (2048)
_BF16 = ml_dtypes.bfloat16


def _build_nc():
    import concourse.bacc as bacc
    import concourse.mybir as mybir
    import concourse.tile as tile
    from concourse import bass_isa

    dt = mybir.dt
    F32, BF16, F16 = dt.float32, dt.bfloat16, dt.float16
    AF = mybir.ActivationFunctionType
    ALU = mybir.AluOpType

    nc = bacc.Bacc("TRN2", target_bir_lowering=False, debug=False,
                   num_devices=_NCORES)

    def din(name, shape, dtype):
        return nc.dram_tensor(name, shape, dtype, kind="ExternalInput").ap()

    hs_l = din("hs_l", [128, 16 * 4096], BF16)     # hsT h-blocked, chunk-permuted
    wq_l = din("wq_l", [128, 16 * 512], BF16)      # wqT h-blocked (pre-scaled)
    wk_l = din("wk_l", [128, 16 * 128], BF16)
    wv_l = din("wv_l", [128, 16 * 128], BF16)
    wo_l = din("wo_l", [128, 4 * 2048], BF16)      # woT hd-blocked
    cos_k = din("cos_k", [128, 4096], F16)         # chunk-permuted like hs
    sinm_k = din("sinm_k", [128, 4096], F16)
    ident = din("ident", [128, 128], BF16)
    outT = nc.dram_tensor("outT", [2048, 2048], BF16, kind="ExternalOutput").ap()

    with tile.TileContext(nc) as tc:
        with (
            tc.tile_pool(name="wp", bufs=1) as wp,
            tc.tile_pool(name="bigp", bufs=1) as bigp,
            tc.tile_pool(name="hsp", bufs=3) as hsp,
            tc.tile_pool(name="vtt", bufs=2) as vttp,
            tc.tile_pool(name="ptp", bufs=4) as ptp,
            tc.tile_pool(name="accp", bufs=2) as accp,
            tc.tile_pool(name="arp", bufs=2) as arp,
            tc.tile_pool(name="atp", bufs=8) as atp,
            tc.tile_pool(name="rbp", bufs=1) as rbp,
            tc.tile_pool(name="rip", bufs=1) as rip,
            tc.tile_pool(name="outp", bufs=3) as outp,
            tc.tile_pool(name="tmpp", bufs=1) as tmpp,
            # PSUM: psS 2x[128,1024] (4 banks) + psAV 1x[128,1024] (2 banks)
            # + psX 2x[128,512] (2 banks) = 8 banks
            tc.tile_pool(name="psS", bufs=2, space="PSUM") as psS,
            tc.tile_pool(name="psAV", bufs=1, space="PSUM") as psAV,
            tc.tile_pool(name="psX", bufs=2, space="PSUM") as psX,
        ):
            # ---- early weights (K/V proj + transposes + rope-k)
            wk_sb = wp.tile([128, 16 * 128], BF16, name="wk_sb")
            nc.sync.dma_start(wk_sb[:, :], wk_l[:, :])
            wv_sb = wp.tile([128, 16 * 128], BF16, name="wv_sb")
            nc.sync.dma_start(wv_sb[:, :], wv_l[:, :])
            id_sb = wp.tile([128, 128], BF16, name="id_sb")
            nc.sync.dma_start(id_sb[:, :], ident[:, :])
            cosk_sb = wp.tile([128, 4096], F16, name="cosk_sb")
            sinmk_sb = wp.tile([128, 4096], F16, name="sinmk_sb")
            wq_sb = wp.tile([128, 16 * 512], BF16, name="wq_sb")
            wo_sb = wp.tile([128, 4 * 2048], BF16, name="wo_sb")

            # ---- persistent activations
            qr = bigp.tile([128, 4 * 2048], BF16, name="qr")    # [d, qh*2048+sq]
            kr = bigp.tile([128, 4096], BF16, name="kr")        # [d, sk]
            vsb = bigp.tile([128, 4096], BF16, name="vsb")      # [sk%128, jt*128+d]

            hs3 = hs_l.rearrange("p (t s) -> p t s", t=16)

            def rope(dst, ps, c0, w):
                # dst = ps * cos + swap_halves(ps) * sinm  (partition dim = d)
                t1 = tmpp.tile([128, 1024], F32, name="t1", tag="t1")
                t2 = tmpp.tile([128, 1024], F32, name="t2", tag="t2")
                nc.vector.tensor_mul(t1[:, :w], ps[:, :w], cosk_sb[:, c0:c0 + w])
                nc.vector.tensor_mul(t2[0:64, :w], ps[64:128, :w],
                                     sinmk_sb[0:64, c0:c0 + w])
                nc.vector.tensor_mul(t2[64:128, :w], ps[0:64, :w],
                                     sinmk_sb[64:128, c0:c0 + w])
                nc.vector.tensor_add(dst, t1[:, :w], t2[:, :w])

            def load_hst(c):
                # hst chunk c: [128, 16, 512] covering kr cols [512c, 512c+512)
                hst = hsp.tile([128, 16 * 512], BF16, name="hst", tag="hst")
                nc.sync.dma_start(
                    hst.rearrange("p (t s) -> p t s", t=16),
                    hs3[:, :, c * 512:(c + 1) * 512],
                )
                return hst

            def q_chunk(hst, qd, qcol):
                # Q proj for one head-dim block over one 512-col chunk
                psq = psX.tile([128, 512], F32, name="psq", tag="psX")
                for ht in range(16):
                    nc.tensor.matmul(
                        psq[:, :],
                        wq_sb[:, ht * 512 + qd * 128: ht * 512 + (qd + 1) * 128],
                        hst[:, ht * 512:(ht + 1) * 512],
                        start=(ht == 0), stop=(ht == 15))
                rope(qr[:, qd * 2048 + qcol: qd * 2048 + qcol + 512],
                     psq, 2048 + qcol, 512)

            # ---- proj phase: pairs of 512-chunks
            def proj_pair(p, q_b0):
                hst_a, hst_b = hs_tiles[2 * p], hs_tiles[2 * p + 1]
                kc0 = p * 1024
                psk = psS.tile([128, 1024], F32, name="psk", tag="psS")
                psv = psS.tile([128, 1024], F32, name="psv", tag="psS")
                vt = vttp.tile([128, 1024], BF16, name="vt", tag="vt")
                for half, hst in ((0, hst_a), (1, hst_b)):
                    sl = slice(half * 512, (half + 1) * 512)
                    for ht in range(16):
                        nc.tensor.matmul(
                            psk[:, sl],
                            wk_sb[:, ht * 128:(ht + 1) * 128],
                            hst[:, ht * 512:(ht + 1) * 512],
                            start=(ht == 0), stop=(ht == 15))
                    for ht in range(16):
                        nc.tensor.matmul(
                            psv[:, sl],
                            wv_sb[:, ht * 128:(ht + 1) * 128],
                            hst[:, ht * 512:(ht + 1) * 512],
                            start=(ht == 0), stop=(ht == 15))
                    nc.scalar.copy(vt[:, sl], psv[:, sl])
                    for j in range(4):  # VT[d, s] -> V[s, d] via PE transpose
                        pst = psX.tile([128, 128], BF16, name="pst", tag="psX")
                        nc.tensor.transpose(
                            pst[:, :], vt[:, half * 512 + j * 128:
                                          half * 512 + (j + 1) * 128],
                            id_sb[:, :])
                        jt = p * 8 + half * 4 + j
                        nc.vector.tensor_copy(vsb[:, jt * 128:(jt + 1) * 128],
                                              pst[:, :])
                    if q_b0:
                        for qd in range(4):
                            q_chunk(hst, qd, (2 * p - 4) * 512 + half * 512)
                rope(kr[:, kc0:kc0 + 1024], psk, kc0, 1024)

            # DMA issue order on the SP ring controls transfer order: first
            # pair's hs chunks, then rope tables, next chunks, then wq/wo.
            hs_tiles = {}
            hs_tiles[0] = load_hst(0)
            hs_tiles[1] = load_hst(1)
            nc.sync.dma_start(cosk_sb[:, :], cos_k[:, :])
            nc.sync.dma_start(sinmk_sb[:, :], sinm_k[:, :])
            hs_tiles[2] = load_hst(2)
            hs_tiles[3] = load_hst(3)
            nc.sync.dma_start(wq_sb[:, :], wq_l[:, :])
            proj_pair(0, False)
            hs_tiles[4] = load_hst(4)
            hs_tiles[5] = load_hst(5)
            proj_pair(1, False)
            hs_tiles[6] = load_hst(6)
            hs_tiles[7] = load_hst(7)
            nc.sync.dma_start(wo_sb[:, :], wo_l[:, :])
            proj_pair(2, True)               # own chunks 4,5 -> Q block b0
            proj_pair(3, False)              # own chunks 6,7 -> Q deferred
            hs_last = (hs_tiles[6], hs_tiles[7])

            # ---- deferred work generators ----------------------------------
            def deferred_q_items():
                # Q proj for own chunks 6,7 (q cols 1024..2048): 8 bursts of
                # one q_chunk each, one MM per yield + one rope yield.
                for qd in range(4):
                    for half in range(2):
                        hst = hs_last[half]
                        qcol = 1024 + half * 512
                        psq = psX.tile([128, 512], F32, name="psqd", tag="psX")
                        for ht in range(16):
                            yield lambda psq=psq, qd=qd, ht=ht, hst=hst: \
                                nc.tensor.matmul(
                                    psq[:, :],
                                    wq_sb[:, ht * 512 + qd * 128:
                                          ht * 512 + (qd + 1) * 128],
                                    hst[:, ht * 512:(ht + 1) * 512],
                                    start=(ht == 0), stop=(ht == 15))

                        def _rope(psq=psq, qd=qd, qcol=qcol):
                            rope(qr[:, qd * 2048 + qcol: qd * 2048 + qcol + 512],
                                 psq, 2048 + qcol, 512)
                        yield _rope

            at_n = {}

            def oproj_items(b):
                # o_proj for block b: 32 PSUM groups of 4 MMs
                for ot in range(16):
                    for half in range(2):
                        def _grp(ot=ot, half=half, b=b):
                            pso = psX.tile([128, 512], F32, name="pso",
                                           tag="psX")
                            for qh in range(4):
                                nc.tensor.matmul(
                                    pso[:, :],
                                    wo_sb[:, qh * 2048 + ot * 128:
                                          qh * 2048 + (ot + 1) * 128],
                                    at_n[4 * b + qh][:, half * 512:
                                                     (half + 1) * 512],
                                    start=(qh == 0), stop=(qh == 3))
                            osb = outp.tile([128, 512], BF16, name="osb",
                                            tag="osb")
                            nc.vector.tensor_copy(osb[:, :], pso[:, :])
                            nc.sync.dma_start(
                                outT[ot * 128:(ot + 1) * 128,
                                     b * 1024 + half * 512:
                                     b * 1024 + half * 512 + 512],
                                osb[:, :])
                        yield _grp

            # ---- attention ------------------------------------------------
            epi = [None]

            def flush_epi():
                if epi[0] is not None:
                    if epi[0]():
                        epi[0] = None

            for b in range(2):
                bg = deferred_q_items() if b == 0 else oproj_items(0)
                nth = 1 if b == 0 else 4     # issue one item every nth jt
                cnt = 0
                for qh in range(4):
                    qsl = qr[:, qh * 2048 + b * 1024: qh * 2048 + (b + 1) * 1024]
                    acc = accp.tile([128, 1024], BF16, name="acc", tag="acc")
                    psav = psAV.tile([128, 1024], F32, name="psav", tag="psAV")
                    for jt in range(32):
                        pss = psS.tile([128, 1024], F32, name="pss", tag="psS")
                        kt = kr[:, jt * 128:(jt + 1) * 128]
                        nc.tensor.matmul(pss[:, 0:512], kt, qsl[:, 0:512],
                                         start=True, stop=True)
                        nc.tensor.matmul(pss[:, 512:1024], kt, qsl[:, 512:1024],
                                         start=True, stop=True)
                        if jt == 0:
                            # exp(jt0) writes straight into acc: initializes
                            # the prob accumulator with no extra copy.
                            pt = acc
                        else:
                            pt = ptp.tile([128, 1024], BF16, name="pt", tag="pt")
                        nc.scalar.activation(pt[:, :], pss[:, :], AF.Exp)
                        vt_ = vsb[:, jt * 128:(jt + 1) * 128]
                        nc.tensor.matmul(psav[:, 0:512], vt_, pt[:, 0:512],
                                         start=(jt == 0), stop=(jt == 31))
                        nc.tensor.matmul(psav[:, 512:1024], vt_, pt[:, 512:1024],
                                         start=(jt == 0), stop=(jt == 31))
                        if jt > 0:
                            nc.vector.tensor_add(acc[:, :], acc[:, :], pt[:, :])
                        if jt == 6 or jt == 12:
                            # deferred reciprocal (jt6) / normalize (jt12) of
                            # the previous head: by jt6 its
                            # partition_all_reduce has finished, so this no
                            # longer blocks the DVE FIFO.
                            flush_epi()
                        cnt += 1
                        if (cnt % nth == 0) and (b == 0 or cnt >= 16):
                            item = next(bg, None)
                            if item is not None:
                                item()
                    # head epilogue: free psav (on ACT: idle at boundaries),
                    # start denom reduce; defer reciprocal + normalize into
                    # the next head's jt loop.
                    araw = arp.tile([128, 1024], BF16, name="araw", tag="araw")
                    nc.scalar.copy(araw[:, :], psav[:, :])
                    rb = rbp.tile([128, 1024], F32, name="rb", tag="rb")
                    nc.gpsimd.partition_all_reduce(
                        rb[:, :], acc[:, :], 128, bass_isa.ReduceOp.add)

                    def mk_epi(rb=rb, araw=araw, qh=qh, b=b):
                        st = {"n": 0}

                        def f():
                            if st["n"] == 0:
                                st["rinv"] = rip.tile([128, 1024], F32,
                                                      name="rinv", tag="rinv")
                                nc.vector.reciprocal_approx_fast(
                                    st["rinv"][:, :], rb[:, :])
                                st["n"] = 1
                                return False
                            at = atp.tile([128, 1024], BF16, name="at", tag="at")
                            nc.vector.tensor_mul(at[:, :], araw[:, :],
                                                 st["rinv"][:, :])
                            at_n[4 * b + qh] = at
                            return True
                        return f
                    epi[0] = mk_epi()
                # drain any remaining background items for this block
                for item in bg:
                    item()

            # tail: o_proj for block 1
            flush_epi()
            flush_epi()
            for item in oproj_items(1):
                item()

    nc.compile()
    return nc


def _blocks_p(x):
    """[(T*128), C] row-major -> [128, T*C] with block t at cols [t*C,(t+1)*C)."""
    t = x.shape[0] // 128
    return np.ascontiguousarray(
        x.reshape(t, 128, -1).transpose(1, 0, 2).reshape(128, -1))


def _prepare_in_maps(hidden_states, wq, wk, wv, wo):
    hs = np.ascontiguousarray(np.asarray(hidden_states, np.float32)[0])  # [S,H]
    hsT = np.ascontiguousarray(hs.T)                                     # [H,S]
    hsT_b = hsT.astype(_BF16)

    inv_freq = 1.0 / (10000.0 ** (np.arange(0, _HD, 2, dtype=np.float32) / _HD))
    t = np.arange(_S, dtype=np.float32)
    freqs = np.einsum("i,j->ij", t, inv_freq)
    emb = np.concatenate([freqs, freqs], axis=-1)                        # [S,128]
    cosT = np.cos(emb).T.astype(np.float16)                               # [128,S]
    sinm = np.sin(emb).astype(np.float32)
    sinm[:, :64] *= -1.0
    sinmT = sinm.T.astype(np.float16)

    scale = 1.0 / math.sqrt(_HD)
    wq = np.asarray(wq, np.float32)
    wk = np.asarray(wk, np.float32)
    wv = np.asarray(wv, np.float32)
    wo = np.asarray(wo, np.float32)

    ident = np.eye(128, dtype=np.float32).astype(_BF16)

    in_maps = []
    for c in range(_NCORES):
        g, sh = c // 2, c % 2
        # hs chunk order: other half first (kr cols 0..2048), own half last
        oh = 1 - sh
        perm = np.concatenate(
            [np.arange(oh * _SQ, (oh + 1) * _SQ),
             np.arange(sh * _SQ, (sh + 1) * _SQ)])
        in_maps.append({
            "hs_l": _blocks_p(np.ascontiguousarray(hsT_b[:, perm])),
            "wq_l": _blocks_p(
                (wq[512 * g:512 * (g + 1), :].T * scale).astype(_BF16)),
            "wk_l": _blocks_p(wk[128 * g:128 * (g + 1), :].T.astype(_BF16)),
            "wv_l": _blocks_p(wv[128 * g:128 * (g + 1), :].T.astype(_BF16)),
            "wo_l": _blocks_p(
                np.ascontiguousarray(wo[:, 512 * g:512 * (g + 1)].T).astype(_BF16)),
            "cos_k": np.ascontiguousarray(cosT[:, perm]),
            "sinm_k": np.ascontiguousarray(sinmT[:, perm]),
            "ident": ident,
        })
    return in_maps


def _run(inputs, trace=False):
    from concourse.bass_utils import run_bass_kernel_spmd

    nc = _build_nc()
    in_maps = _prepare_in_maps(**inputs)
    res = run_bass_kernel_spmd(nc, in_maps, core_ids=list(range(_NCORES)),
                               trace=trace)
    halves = []
    for sh in range(2):
        acc = np.zeros((2048, 2048), np.float32)
        for g in range(4):
            acc += np.asarray(res.results[2 * g + sh]["outT"], dtype=np.float32)
        halves.append(acc.T)
    out = np.concatenate(halves, axis=0)[None]
    return np.ascontiguousarray(out, dtype=np.float32), res


def kernel(**inputs):
    out, _ = _run(inputs, trace=False)
    return out


# revision 18
# speedup vs baseline: 1.1640x; 1.1640x over previous
"""Llama-style GQA attention (S=4096, H=2048, 16 q heads / 4 kv heads, d=128, fp32)
on 8 Trainium2 NeuronCores.

Sharding: 4 head-groups x 2 sequence-halves. Core c = 2*g + sh owns q heads
[4g, 4g+4) (one kv head g) and query rows [2048*sh, 2048*(sh+1)). Each core
computes its partial o_proj output transposed ([out_feat, seq], bf16); the host
sums the 4 head-group partials per sequence half and concatenates.

v2 schedule (single pass over hs, ACT-paced attention inner loop):
  hs chunk order per core: OTHER seq-half first (kr/vsb cols 0..2048), own
  half last (cols 2048..4096) -> rope tables for q are cos_k[:, 2048+q].
  proj: stream hs in 512-col chunks (pairs share [128,1024] PSUM tiles);
        K/V projected for all 8 chunks, Q only for own chunks 0,1 (block b0);
        Q for chunks 2,3 (block b1) is deferred and drip-fed into attention
        block b0 through the psX PSUM ring.
  attn: per (block, head): 32 key-tiles; each jt issues
        scoresT MM (2x512) -> ACT exp [128,1024] -> PV MM (2x512, PSUM accum)
        -> DVE 4x-mode prob accumulate, plus one deferred PE item (Q proj MM
        for b0, or a 4-MM o_proj group every 4th jt for b1). Denominator =
        GpSimd partition_all_reduce; 1/x via DVE reciprocal_approx_fast;
        PSUM attn accumulator freed early via GpSimd bf16 copy; o_proj
        outputs copied bf16 on GpSimd and DMA'd out per 512-col group.
"""

import math

import numpy as np
import ml_dtypes

_S, _H, _HD = 4096, 2048, 128
_NCORES = 8
_SQ = _S // 2          # per-core query rows (2048)
_BF16 = ml_dtypes.bfloat16


def _build_nc():
    import concourse.bacc as bacc
    import concourse.mybir as mybir
    import concourse.tile as tile
    from concourse import bass_isa

    dt = mybir.dt
    F32, BF16, F16 = dt.float32, dt.bfloat16, dt.float16
    AF = mybir.ActivationFunctionType
    ALU = mybir.AluOpType

    nc = bacc.Bacc("TRN2", target_bir_lowering=False, debug=False,
                   num_devices=_NCORES)

    def din(name, shape, dtype):
        return nc.dram_tensor(name, shape, dtype, kind="ExternalInput").ap()

    hs_l = din("hs_l", [128, 16 * 4096], BF16)     # hsT h-blocked, chunk-permuted
    wq_l = din("wq_l", [128, 16 * 512], BF16)      # wqT h-blocked (pre-scaled)
    wk_l = din("wk_l", [128, 16 * 128], BF16)
    wv_l = din("wv_l", [128, 16 * 128], BF16)
    wo_l = din("wo_l", [128, 4 * 2048], BF16)      # woT hd-blocked
    cos_k = din("cos_k", [128, 4096], F16)         # chunk-permuted like hs
    sinm_k = din("sinm_k", [128, 4096], F16)
    ident = din("ident", [128, 128], BF16)
    outT = nc.dram_tensor("outT", [2048, 2048], BF16, kind="ExternalOutput").ap()

    with tile.TileContext(nc) as tc:
        with (
            tc.tile_pool(name="wp", bufs=1) as wp,
            tc.tile_pool(name="bigp", bufs=1) as bigp,
            tc.tile_pool(name="hsp", bufs=3) as hsp,
            tc.tile_pool(name="vtt", bufs=2) as vttp,
            tc.tile_pool(name="ptp", bufs=4) as ptp,
            tc.tile_pool(name="accp", bufs=2) as accp,
            tc.tile_pool(name="arp", bufs=2) as arp,
            tc.tile_pool(name="atp", bufs=8) as atp,
            tc.tile_pool(name="rbp", bufs=1) as rbp,
            tc.tile_pool(name="rip", bufs=1) as rip,
            tc.tile_pool(name="outp", bufs=3) as outp,
            tc.tile_pool(name="tmpp", bufs=1) as tmpp,
            # PSUM: psS 2x[128,1024] (4 banks) + psAV 1x[128,1024] (2 banks)
            # + psX 2x[128,512] (2 banks) = 8 banks
            tc.tile_pool(name="psS", bufs=2, space="PSUM") as psS,
            tc.tile_pool(name="psAV", bufs=1, space="PSUM") as psAV,
            tc.tile_pool(name="psX", bufs=2, space="PSUM") as psX,
        ):
            # ---- early weights (K/V proj + transposes + rope-k)
            wk_sb = wp.tile([128, 16 * 128], BF16, name="wk_sb")
            nc.sync.dma_start(wk_sb[:, :], wk_l[:, :])
            wv_sb = wp.tile([128, 16 * 128], BF16, name="wv_sb")
            nc.sync.dma_start(wv_sb[:, :], wv_l[:, :])
            id_sb = wp.tile([128, 128], BF16, name="id_sb")
            nc.sync.dma_start(id_sb[:, :], ident[:, :])
            cosk_sb = wp.tile([128, 4096], F16, name="cosk_sb")
            sinmk_sb = wp.tile([128, 4096], F16, name="sinmk_sb")
            wq_sb = wp.tile([128, 16 * 512], BF16, name="wq_sb")
            wo_sb = wp.tile([128, 4 * 2048], BF16, name="wo_sb")

            # ---- persistent activations
            qr = bigp.tile([128, 4 * 2048], BF16, name="qr")    # [d, qh*2048+sq]
            kr = bigp.tile([128, 4096], BF16, name="kr")        # [d, sk]
            vsb = bigp.tile([128, 4096], BF16, name="vsb")      # [sk%128, jt*128+d]

            hs3 = hs_l.rearrange("p (t s) -> p t s", t=16)

            def rope(dst, ps, c0, w):
                # dst = ps * cos + swap_halves(ps) * sinm  (partition dim = d)
                t1 = tmpp.tile([128, 1024], F32, name="t1", tag="t1")
                t2 = tmpp.tile([128, 1024], F32, name="t2", tag="t2")
                nc.vector.tensor_mul(t1[:, :w], ps[:, :w], cosk_sb[:, c0:c0 + w])
                nc.vector.tensor_mul(t2[0:64, :w], ps[64:128, :w],
                                     sinmk_sb[0:64, c0:c0 + w])
                nc.vector.tensor_mul(t2[64:128, :w], ps[0:64, :w],
                                     sinmk_sb[64:128, c0:c0 + w])
                nc.vector.tensor_add(dst, t1[:, :w], t2[:, :w])

            def load_hst(c):
                # hst chunk c: [128, 16, 512] covering kr cols [512c, 512c+512)
                hst = hsp.tile([128, 16 * 512], BF16, name="hst", tag="hst")
                nc.sync.dma_start(
                    hst.rearrange("p (t s) -> p t s", t=16),
                    hs3[:, :, c * 512:(c + 1) * 512],
                )
                return hst

            def q_chunk(hst, qd, qcol):
                # Q proj for one head-dim block over one 512-col chunk
                psq = psX.tile([128, 512], F32, name="psq", tag="psX")
                for ht in range(16):
                    nc.tensor.matmul(
                        psq[:, :],
                        wq_sb[:, ht * 512 + qd * 128: ht * 512 + (qd + 1) * 128],
                        hst[:, ht * 512:(ht + 1) * 512],
                        start=(ht == 0), stop=(ht == 15))
                rope(qr[:, qd * 2048 + qcol: qd * 2048 + qcol + 512],
                     psq, 2048 + qcol, 512)

            # ---- proj phase: pairs of 512-chunks
            def proj_pair(p, q_b0):
                hst_a, hst_b = hs_tiles[2 * p], hs_tiles[2 * p + 1]
                kc0 = p * 1024
                psk = psS.tile([128, 1024], F32, name="psk", tag="psS")
                psv = psS.tile([128, 1024], F32, name="psv", tag="psS")
                vt = vttp.tile([128, 1024], BF16, name="vt", tag="vt")
                for half, hst in ((0, hst_a), (1, hst_b)):
                    sl = slice(half * 512, (half + 1) * 512)
                    for ht in range(16):
                        nc.tensor.matmul(
                            psk[:, sl],
                            wk_sb[:, ht * 128:(ht + 1) * 128],
                            hst[:, ht * 512:(ht + 1) * 512],
                            start=(ht == 0), stop=(ht == 15))
                    for ht in range(16):
                        nc.tensor.matmul(
                            psv[:, sl],
                            wv_sb[:, ht * 128:(ht + 1) * 128],
                            hst[:, ht * 512:(ht + 1) * 512],
                            start=(ht == 0), stop=(ht == 15))
                    nc.scalar.copy(vt[:, sl], psv[:, sl])
                    for j in range(4):  # VT[d, s] -> V[s, d] via PE transpose
                        pst = psX.tile([128, 128], BF16, name="pst", tag="psX")
                        nc.tensor.transpose(
                            pst[:, :], vt[:, half * 512 + j * 128:
                                          half * 512 + (j + 1) * 128],
                            id_sb[:, :])
                        jt = p * 8 + half * 4 + j
                        nc.vector.tensor_copy(vsb[:, jt * 128:(jt + 1) * 128],
                                              pst[:, :])
                    if q_b0:
                        for qd in range(4):
                            q_chunk(hst, qd, (2 * p - 4) * 512 + half * 512)
                rope(kr[:, kc0:kc0 + 1024], psk, kc0, 1024)

            # DMA issue order on the SP ring controls transfer order: first
            # pair's hs chunks, then rope tables, next chunks, then wq/wo.
            hs_tiles = {}
            hs_tiles[0] = load_hst(0)
            hs_tiles[1] = load_hst(1)
            nc.sync.dma_start(cosk_sb[:, :], cos_k[:, :])
            nc.sync.dma_start(sinmk_sb[:, :], sinm_k[:, :])
            hs_tiles[2] = load_hst(2)
            hs_tiles[3] = load_hst(3)
            nc.sync.dma_start(wq_sb[:, :], wq_l[:, :])
            proj_pair(0, False)
            hs_tiles[4] = load_hst(4)
            hs_tiles[5] = load_hst(5)
            proj_pair(1, False)
            hs_tiles[6] = load_hst(6)
            hs_tiles[7] = load_hst(7)
            nc.sync.dma_start(wo_sb[:, :], wo_l[:, :])
            proj_pair(2, True)               # own chunks 4,5 -> Q block b0
            proj_pair(3, False)              # own chunks 6,7 -> Q deferred
            hs_last = (hs_tiles[6], hs_tiles[7])

            # ---- deferred work generators ----------------------------------
            def deferred_q_items():
                # Q proj for own chunks 6,7 (q cols 1024..2048): 8 bursts of
                # one q_chunk each, one MM per yield + one rope yield.
                for qd in range(4):
                    for half in range(2):
                        hst = hs_last[half]
                        qcol = 1024 + half * 512
                        psq = psX.tile([128, 512], F32, name="psqd", tag="psX")
                        for ht in range(16):
                            yield lambda psq=psq, qd=qd, ht=ht, hst=hst: \
                                nc.tensor.matmul(
                                    psq[:, :],
                                    wq_sb[:, ht * 512 + qd * 128:
                                          ht * 512 + (qd + 1) * 128],
                                    hst[:, ht * 512:(ht + 1) * 512],
                                    start=(ht == 0), stop=(ht == 15))

                        def _rope(psq=psq, qd=qd, qcol=qcol):
                            rope(qr[:, qd * 2048 + qcol: qd * 2048 + qcol + 512],
                                 psq, 2048 + qcol, 512)
                        yield _rope

            at_n = {}

            def oproj_items(b):
                # o_proj for block b: 32 PSUM groups of 4 MMs
                for ot in range(16):
                    for half in range(2):
                        def _grp(ot=ot, half=half, b=b):
                            pso = psX.tile([128, 512], F32, name="pso",
                                           tag="psX")
                            for qh in range(4):
                                nc.tensor.matmul(
                                    pso[:, :],
                                    wo_sb[:, qh * 2048 + ot * 128:
                                          qh * 2048 + (ot + 1) * 128],
                                    at_n[4 * b + qh][:, half * 512:
                                                     (half + 1) * 512],
                                    start=(qh == 0), stop=(qh == 3))
                            osb = outp.tile([128, 512], BF16, name="osb",
                                            tag="osb")
                            nc.vector.tensor_copy(osb[:, :], pso[:, :])
                            nc.sync.dma_start(
                                outT[ot * 128:(ot + 1) * 128,
                                     b * 1024 + half * 512:
                                     b * 1024 + half * 512 + 512],
                                osb[:, :])
                        yield _grp

            # ---- attention ------------------------------------------------
            epi = [None]

            def flush_epi():
                if epi[0] is not None:
                    if epi[0]():
                        epi[0] = None

            def score_mm(qsl, jt):
                pss = psS.tile([128, 1024], F32, name="pss", tag="psS")
                kt = kr[:, jt * 128:(jt + 1) * 128]
                nc.tensor.matmul(pss[:, 0:512], kt, qsl[:, 0:512],
                                 start=True, stop=True)
                nc.tensor.matmul(pss[:, 512:1024], kt, qsl[:, 512:1024],
                                 start=True, stop=True)
                return pss

            def qsl_of(b, qh):
                return qr[:, qh * 2048 + b * 1024: qh * 2048 + (b + 1) * 1024]

            pss_next = [None]
            for b in range(2):
                bg = deferred_q_items() if b == 0 else oproj_items(0)
                nth = 1 if b == 0 else 4     # issue one item every nth jt
                cnt = 0
                for qh in range(4):
                    qsl = qsl_of(b, qh)
                    acc = accp.tile([128, 1024], BF16, name="acc", tag="acc")
                    psav = psAV.tile([128, 1024], F32, name="psav", tag="psAV")
                    for jt in range(32):
                        if jt == 0 and pss_next[0] is not None:
                            pss = pss_next[0]
                            pss_next[0] = None
                        else:
                            pss = score_mm(qsl, jt)
                        if jt == 31 and not (b == 1 and qh == 3):
                            # prefetch next head's first score tile so the
                            # exp stream never starves across the boundary
                            nb, nqh = (b, qh + 1) if qh < 3 else (b + 1, 0)
                            pss_next[0] = score_mm(qsl_of(nb, nqh), 0)
                        if jt == 0:
                            # exp(jt0) writes straight into acc: initializes
                            # the prob accumulator with no extra copy.
                            pt = acc
                        else:
                            pt = ptp.tile([128, 1024], BF16, name="pt", tag="pt")
                        nc.scalar.activation(pt[:, :], pss[:, :], AF.Exp)
                        vt_ = vsb[:, jt * 128:(jt + 1) * 128]
                        nc.tensor.matmul(psav[:, 0:512], vt_, pt[:, 0:512],
                                         start=(jt == 0), stop=(jt == 31))
                        nc.tensor.matmul(psav[:, 512:1024], vt_, pt[:, 512:1024],
                                         start=(jt == 0), stop=(jt == 31))
                        if jt > 0:
                            nc.vector.tensor_add(acc[:, :], acc[:, :], pt[:, :])
                        if jt == 16 or jt == 24:
                            # deferred reciprocal (jt16) / normalize (jt24) of
                            # the previous head: its partition_all_reduce has
                            # long finished, so this never blocks the DVE FIFO.
                            flush_epi()
                        cnt += 1
                        if (cnt % nth == 0) and (b == 0 or cnt >= 28):
                            item = next(bg, None)
                            if item is not None:
                                item()
                    # head epilogue: free psav (on ACT: idle at boundaries),
                    # start denom reduce; defer reciprocal + normalize into
                    # the next head's jt loop.
                    araw = arp.tile([128, 1024], BF16, name="araw", tag="araw")
                    nc.scalar.copy(araw[:, :], psav[:, :])
                    rb = rbp.tile([128, 1024], F32, name="rb", tag="rb")
                    nc.gpsimd.partition_all_reduce(
                        rb[:, :], acc[:, :], 128, bass_isa.ReduceOp.add)

                    def mk_epi(rb=rb, araw=araw, qh=qh, b=b):
                        st = {"n": 0}

                        def f():
                            if st["n"] == 0:
                                st["rinv"] = rip.tile([128, 1024], F32,
                                                      name="rinv", tag="rinv")
                                nc.vector.reciprocal_approx_fast(
                                    st["rinv"][:, :], rb[:, :])
                                st["n"] = 1
                                return False
                            at = atp.tile([128, 1024], BF16, name="at", tag="at")
                            nc.vector.tensor_mul(at[:, :], araw[:, :],
                                                 st["rinv"][:, :])
                            at_n[4 * b + qh] = at
                            return True
                        return f
                    epi[0] = mk_epi()
                # drain any remaining background items for this block
                for item in bg:
                    item()

            # tail: o_proj for block 1
            flush_epi()
            flush_epi()
            for item in oproj_items(1):
                item()

    nc.compile()
    return nc


def _blocks_p(x):
    """[(T*128), C] row-major -> [128, T*C] with block t at cols [t*C,(t+1)*C)."""
    t = x.shape[0] // 128
    return np.ascontiguousarray(
        x.reshape(t, 128, -1).transpose(1, 0, 2).reshape(128, -1))


def _prepare_in_maps(hidden_states, wq, wk, wv, wo):
    hs = np.ascontiguousarray(np.asarray(hidden_states, np.float32)[0])  # [S,H]
    hsT = np.ascontiguousarray(hs.T)                                     # [H,S]
    hsT_b = hsT.astype(_BF16)

    inv_freq = 1.0 / (10000.0 ** (np.arange(0, _HD, 2, dtype=np.float32) / _HD))
    t = np.arange(_S, dtype=np.float32)
    freqs = np.einsum("i,j->ij", t, inv_freq)
    emb = np.concatenate([freqs, freqs], axis=-1)                        # [S,128]
    cosT = np.cos(emb).T.astype(np.float16)                               # [128,S]
    sinm = np.sin(emb).astype(np.float32)
    sinm[:, :64] *= -1.0
    sinmT = sinm.T.astype(np.float16)

    scale = 1.0 / math.sqrt(_HD)
    wq = np.asarray(wq, np.float32)
    wk = np.asarray(wk, np.float32)
    wv = np.asarray(wv, np.float32)
    wo = np.asarray(wo, np.float32)

    ident = np.eye(128, dtype=np.float32).astype(_BF16)

    in_maps = []
    for c in range(_NCORES):
        g, sh = c // 2, c % 2
        # hs chunk order: other half first (kr cols 0..2048), own half last
        oh = 1 - sh
        perm = np.concatenate(
            [np.arange(oh * _SQ, (oh + 1) * _SQ),
             np.arange(sh * _SQ, (sh + 1) * _SQ)])
        in_maps.append({
            "hs_l": _blocks_p(np.ascontiguousarray(hsT_b[:, perm])),
            "wq_l": _blocks_p(
                (wq[512 * g:512 * (g + 1), :].T * scale).astype(_BF16)),
            "wk_l": _blocks_p(wk[128 * g:128 * (g + 1), :].T.astype(_BF16)),
            "wv_l": _blocks_p(wv[128 * g:128 * (g + 1), :].T.astype(_BF16)),
            "wo_l": _blocks_p(
                np.ascontiguousarray(wo[:, 512 * g:512 * (g + 1)].T).astype(_BF16)),
            "cos_k": np.ascontiguousarray(cosT[:, perm]),
            "sinm_k": np.ascontiguousarray(sinmT[:, perm]),
            "ident": ident,
        })
    return in_maps


def _run(inputs, trace=False):
    from concourse.bass_utils import run_bass_kernel_spmd

    nc = _build_nc()
    in_maps = _prepare_in_maps(**inputs)
    res = run_bass_kernel_spmd(nc, in_maps, core_ids=list(range(_NCORES)),
                               trace=trace)
    halves = []
    for sh in range(2):
        acc = np.zeros((2048, 2048), np.float32)
        for g in range(4):
            acc += np.asarray(res.results[2 * g + sh]["outT"], dtype=np.float32)
        halves.append(acc.T)
    out = np.concatenate(halves, axis=0)[None]
    return np.ascontiguousarray(out, dtype=np.float32), res


def kernel(**inputs):
    out, _ = _run(inputs, trace=False)
    return out
